# revision 2
# baseline (speedup 1.0000x reference)
"""Deformable Conv2d (4,64,160,160) -> (4,64,158,158) on 8 trn2 NeuronCores.

v2: bf16 4-corner single gather from a dual-parity row-pair-interleaved DRAM
layout, fast-mode (2x) DVE bilinear combine with c-pair-replicated weights,
strips via SBUF-source transpose dma_gather (no PE transposes), bf16 convs
with fp32 edge re-pass for snap-boundary precision.

Sharding: core = (sample b = core//2, row-half = core%2); each core computes
79 output rows (12482 px) of one sample.

Layouts (per core):
  raster pixel p in [0, 12482): 40 conv blocks of 316 px (block 39: 158).
  packed col space: block b -> (g = b%7, slot = b//7), col = slot*316 + q.
  packed rows: side*64 + n*7 + g (side 0=x/1=y offsets, n tap, g group).
  px-layout: partition = packed col % 128 for col chunk k in [0,15).
  dual-parity pairs DRAM [26082, 128] bf16: entry (par,pr,y) = rows
  (2pr+par, 2pr+par+1) at col y, channels [row0 c*, row1 c*]; gather elem
  256 vals (step 128) = 4 bilinear corners [colY(2rows), colY+1(2rows)].
  gather idx = (s_x>>1)*162 + s_y + (s_x&1)*13122.
  output cols: (k*7+g)*128 + p  (packed-pixel order; host reassembles).
"""

import numpy as np
import ml_dtypes

import concourse.bass as bass
import concourse.mybir as mybir
import concourse.tile as tile
from concourse.bass import AP

F32 = mybir.dt.float32
BF16 = mybir.dt.bfloat16
I16 = mybir.dt.int16
I32 = mybir.dt.int32
AL = mybir.AluOpType
AF = mybir.ActivationFunctionType

B, C, H, W = 4, 64, 160, 160
Hp, Wp = H + 2, W + 2          # 162
h, w = H - 2, W - 2            # 158
R = h // 2                     # 79 rows per core
P = R * w                      # 12482 px per core
NBLK = 40
NG = 7
CW = 1920                      # packed cols
NK = 15
FULL = NK * 128                # 1920
RW = 42 * 316                  # 13272 raster staging cols
WPITCH = NK * 504              # 7560 wrapped idx pitch
NPAIR_A = 81 * Wp              # 13122
NPAIR = NPAIR_A + 80 * Wp      # 26082
OUTW = NK * 896                # 13440


def _ap(base: AP, extra_off, dims):
    return AP(base.tensor, base.offset + extra_off, dims)


def build_nc():
    from concourse import bacc
    nc = bacc.Bacc("TRN2")

    xpair_cl = nc.dram_tensor("xpair_cl", [NPAIR, 2 * C], BF16, kind="ExternalInput")
    x_sb_in = nc.dram_tensor("x_sb_in", [C, 81 * W], BF16, kind="ExternalInput")
    x4_in = nc.dram_tensor("x4_in", [C, 4 * W], F32, kind="ExternalInput")
    xc_in = nc.dram_tensor("xc_in", [C, 83 * 4], F32, kind="ExternalInput")
    w_taps_in = nc.dram_tensor("w_taps_in", [C, 9 * 128], BF16,
                               kind="ExternalInput")
    w_tapsf_in = nc.dram_tensor("w_tapsf_in", [C, 9 * 128], F32,
                                kind="ExternalInput")
    w_d_in = nc.dram_tensor("w_d_in", [128, 5 * 64], BF16, kind="ExternalInput")
    b_d_in = nc.dram_tensor("b_d_in", [64, 1], F32, kind="ExternalInput")
    base_pk_in = nc.dram_tensor("base_pk_in", [128, CW], F32, kind="ExternalInput")
    ident_in = nc.dram_tensor("ident_in", [128, 128], F32, kind="ExternalInput")
    ws_in = nc.dram_tensor("ws_in", [128, 56], I16, kind="ExternalInput")
    out_d = nc.dram_tensor("out_d", [64, OUTW], F32, kind="ExternalOutput")

    xpair_pairs = AP(xpair_cl[:].tensor, 0, [[2 * C, NPAIR - 1], [1, 4 * C]])

    with tile.TileContext(nc) as tc:
        with tc.tile_pool(name="persist", bufs=1) as pp:
            b_d_t = pp.tile([64, 1], F32, tag="bd")
            nc.sync.dma_start(b_d_t[:], b_d_in[:])
            w_d_t = pp.tile([128, 5 * 64], BF16, tag="wd")
            nc.sync.dma_start(w_d_t[:], w_d_in[:])
            ws_t = pp.tile([128, 56], I16, tag="ws")
            nc.sync.dma_start(ws_t[:], ws_in[:])
            identb = pp.tile([128, 128], BF16, tag="identb")
            nc.gpsimd.dma_start(identb[:], ident_in[:])

            wrapped = pp.tile([128, WPITCH], I16, tag="wrapped")
            wlr = pp.tile([128, 2 * FULL], F32, tag="wlr")
            vw = pp.tile([128, NK * 63 * 2], F32, tag="vw")
            wlrv = wlr[:]
            wl = _ap(wlrv, 0, [wlrv.ap[0], [1, FULL]])
            wr = _ap(wlrv, FULL, [wlrv.ap[0], [1, FULL]])

            with tc.tile_pool(name="mid", bufs=1) as pm:
                ident = pm.tile([128, 128], F32, tag="ident")
                nc.sync.dma_start(ident[:], ident_in[:])
                offs_pk = pm.tile([128, CW], F32, tag="offs_pk")
                nc.gpsimd.memset(offs_pk[:], 0.0)
                ov = offs_pk[:]
                ppk = ov.ap[0][0]

                # ---------------- Phase A: offset conv ----------------
                # packed rows now side*64 + g*9 + n; conv writes offs_pk
                # directly from psum (per-block g-specific lhsT layout).
                with (
                    tc.tile_pool(name="phaseA", bufs=1) as pa,
                    tc.tile_pool(name="psum_conv", bufs=2, space="PSUM") as pconv,
                ):
                    x_sb = pa.tile([C, 81 * W], BF16, tag="x_sb")
                    nc.sync.dma_start(x_sb[:], x_sb_in[:])
                    w_t = pa.tile([C, 9 * 128], BF16, tag="w_taps")
                    nc.sync.dma_start(w_t[:], w_taps_in[:])
                    x4 = pa.tile([C, 4 * W], F32, tag="x4")
                    nc.sync.dma_start(x4[:], x4_in[:])
                    xc = pa.tile([C, 83 * 4], F32, tag="xc")
                    nc.sync.dma_start(xc[:], xc_in[:])
                    w_tf = pa.tile([C, 9 * 128], F32, tag="w_tf")
                    nc.sync.dma_start(w_tf[:], w_tapsf_in[:])

                    stg = [pa.tile([128, 6 * 316], F32, tag=f"stg{g}",
                                   name=f"stg{g}") for g in range(NG)]
                    # unwritten tails (beyond last block per g)
                    nc.gpsimd.memset(stg[4][:, 5 * 316 + 158:6 * 316], 0.0)
                    nc.gpsimd.memset(stg[5][:, 5 * 316:6 * 316], 0.0)
                    nc.gpsimd.memset(stg[6][:, 5 * 316:6 * 316], 0.0)

                    xb = x_sb[:]
                    for blk in range(NBLK):
                        g, slot = blk % NG, blk // NG
                        rows = 2 if blk < NBLK - 1 else 1
                        npx = rows * w
                        ps = pconv.tile([128, 316], F32, tag="psc")
                        for t in range(9):
                            ki, kj = t // 3, t % 3
                            rhs = _ap(xb, (2 * blk + ki) * W + kj,
                                      [xb.ap[0], [W, rows], [1, w]])
                            nc.tensor.matmul(
                                ps[:, 0:npx],
                                w_t[:, t * 128:(t + 1) * 128],
                                rhs, start=(t == 0), stop=(t == 8))
                        for side in range(2):
                            nc.scalar.copy(
                                stg[g][side * 64:side * 64 + 9,
                                       slot * 316:slot * 316 + npx],
                                ps[side * 64:side * 64 + 9, 0:npx])

                    # fp32 edge row-pass (rows 0-1 = block 0, g=0)
                    x4v, xcv = x4[:], xc[:]
                    ps2 = pconv.tile([128, 316], F32, tag="ps_er")
                    for t in range(9):
                        ki, kj = t // 3, t % 3
                        rhs = _ap(x4v, ki * W + kj, [x4v.ap[0], [W, 2], [1, w]])
                        nc.tensor.matmul(
                            ps2[:], w_tf[:, t * 128:(t + 1) * 128], rhs,
                            start=(t == 0), stop=(t == 8))
                    for side in range(2):
                        nc.scalar.copy(
                            stg[0][side * 64:side * 64 + 9, 0:316],
                            ps2[side * 64:side * 64 + 9, :])

                    # fp32 edge col-pass (cols 0-1), per (g, parity)
                    for g in range(NG):
                        for par_ in range(2):
                            T = sum(1 for t in range(6)
                                    if 14 * t + 2 * g + par_ <= 78)
                            ps3 = pconv.tile([128, 12], F32, tag="ps_ec")
                            for t in range(9):
                                ki, kj = t // 3, t % 3
                                rhs = _ap(xcv, (2 * g + par_ + ki) * 4 + kj,
                                          [xcv.ap[0], [56, T], [1, 2]])
                                nc.tensor.matmul(
                                    ps3[:, 0:2 * T],
                                    w_tf[:, t * 128:(t + 1) * 128],
                                    rhs, start=(t == 0), stop=(t == 8))
                            sgv = stg[g][:]
                            sgp = sgv.ap[0][0]
                            p3v = ps3[:]
                            for side in range(2):
                                nc.scalar.copy(
                                    _ap(sgv, side * 64 * sgp + par_ * 158,
                                        [[sgp, 9], [316, T], [1, 2]]),
                                    _ap(p3v, side * 64 * p3v.ap[0][0],
                                        [[p3v.ap[0][0], 9], [2, T], [1, 2]]))

                    # scatter staging -> packed rows: 14 big DMAs
                    for g in range(NG):
                        sgv = stg[g][:]
                        sgp = sgv.ap[0][0]
                        for side in range(2):
                            eng = nc.sync if side == 0 else nc.scalar
                            eng.dma_start(
                                _ap(ov, (side * 64 + g * 9) * ppk,
                                    [[ppk, 9], [1, 6 * 316]]),
                                _ap(sgv, side * 64 * sgp,
                                    [[sgp, 9], [1, 6 * 316]]))

                # ---------------- Phase B ----------------
                with (
                    tc.tile_pool(name="phaseB", bufs=1) as pb,
                    tc.tile_pool(name="pbbig", bufs=4) as pbig,
                    tc.tile_pool(name="pbh", bufs=5) as ph,
                    tc.tile_pool(name="psum_b", bufs=4, space="PSUM") as psb,
                ):
                    pxy = pb.tile([128, CW], F32, tag="b_pxy")
                    fl = pb.tile([128, CW], F32, tag="b_fl")
                    with tc.tile_pool(name="pb1", bufs=1) as p1:
                        # B1: pair-idx pipeline (packed layout)
                        bpk = pbig.tile([128, CW], F32, tag="big")
                        nc.sync.dma_start(bpk[:], base_pk_in[:])
                        nc.vector.tensor_tensor(pxy[:], offs_pk[:], bpk[:], AL.add)
                        ci1 = p1.tile([128, CW], I32, tag="b_ci")
                        nc.vector.tensor_copy(ci1[:], pxy[:])
                        tmp = pbig.tile([128, CW], F32, tag="big")
                        nc.vector.tensor_copy(tmp[:], ci1[:])
                        nc.vector.tensor_tensor(fl[:], tmp[:], pxy[:], AL.is_gt)
                        nc.vector.tensor_tensor(fl[:], tmp[:], fl[:], AL.subtract)
                        s_q = p1.tile([128, CW], F32, tag="b_sq")
                        nc.vector.tensor_scalar(s_q[:], fl[:], 0.0, 160.0,
                                                AL.max, AL.min)
                        sx, sy = s_q[0:64, :], s_q[64:128, :]
                        th = p1.tile([64, CW], F32, tag="b_th")
                        nc.vector.tensor_scalar(th[:], sx, 0.5, None, AL.mult)
                        ci2 = p1.tile([64, CW], I32, tag="b_ci2")
                        nc.vector.tensor_copy(ci2[:], th[:])
                        tm2 = p1.tile([64, CW], F32, tag="b_tm2")
                        nc.vector.tensor_copy(tm2[:], ci2[:])
                        hx = p1.tile([64, CW], F32, tag="b_hx")
                        nc.vector.tensor_tensor(hx[:], tm2[:], th[:], AL.is_gt)
                        nc.vector.tensor_tensor(hx[:], tm2[:], hx[:], AL.subtract)
                        par = p1.tile([64, CW], F32, tag="b_par")
                        nc.vector.scalar_tensor_tensor(
                            par[:], hx[:], -2.0, sx, AL.mult, AL.add)
                        syc = p1.tile([64, CW], F32, tag="b_syc")
                        nc.vector.tensor_copy(syc[:], sy)
                        t3 = p1.tile([64, CW], F32, tag="b_t3")
                        nc.vector.scalar_tensor_tensor(
                            t3[:], hx[:], float(Wp), syc[:], AL.mult, AL.add)
                        idxq = p1.tile([64, CW], F32, tag="b_idxq")
                        nc.vector.scalar_tensor_tensor(
                            idxq[:], par[:], float(NPAIR_A), t3[:], AL.mult,
                            AL.add)

                        # B2: wrapped idx via [64,16] PE transposes, batched
                        # per-k psum + one Act copy + per-k replication
                        idv = idxq[:]
                        wv = wrapped[:]
                        wpp = wv.ap[0][0]
                        for k in range(NK):
                            pst = psb.tile([16, 512], F32, tag="ps_wrap")
                            pv = pst[:]
                            for sl in range(8):
                                nc.tensor.transpose(
                                    _ap(pv, sl * 64, [pv.ap[0], [1, 64]]),
                                    _ap(idv, 16 * (k * 8 + sl),
                                        [idv.ap[0], [1, 16]]),
                                    ident[0:64, 0:64])
                            nc.scalar.copy(
                                _ap(wv, k * 504,
                                    [[wpp, 16], [1, 8], [8, 9], [72, 7]]),
                                _ap(pv, 0,
                                    [[pv.ap[0][0], 16], [64, 8], [1, 9],
                                     [9, 7]]))
                            for i, rep in enumerate((16, 32, 64)):
                                eng = nc.sync if i % 2 == 0 else nc.scalar
                                eng.dma_start(
                                    _ap(wv, k * 504 + rep * wpp,
                                        [[wpp, rep], [1, 504]]),
                                    _ap(wv, k * 504, [[wpp, rep], [1, 504]]))

                    # B3: transpose positions+floors to px layout
                    pT = pb.tile([128, FULL], F32, tag="b_pT")
                    fT = pb.tile([128, FULL], F32, tag="b_fT")
                    for src, dst in ((pxy, pT), (fl, fT)):
                        sv_ = src[:]
                        for k in range(NK):
                            pst2 = psb.tile([128, 128], F32, tag="ps_ot")
                            nc.tensor.transpose(
                                pst2[:], _ap(sv_, 128 * k, [sv_.ap[0], [1, 128]]),
                                ident[:, :])
                            nc.scalar.copy(dst[:, k * 128:(k + 1) * 128], pst2[:])

                    # B4: weights (px layout); p=pT, f=fT
                    c1 = pbig.tile([128, FULL], F32, tag="big")
                    nc.vector.tensor_scalar(c1[:], pT[:], 1.0, None, AL.is_lt)
                    c2 = pbig.tile([128, FULL], F32, tag="big")
                    nc.vector.tensor_scalar(c2[:], pT[:], float(H), None, AL.is_gt)
                    nc.vector.tensor_tensor(c2[:], c1[:], c2[:], AL.max)
                    nc.vector.tensor_tensor(c1[:], fT[:], pT[:], AL.subtract)
                    nc.vector.tensor_tensor(c1[:], c2[:], c1[:], AL.mult)
                    nc.vector.tensor_tensor(pT[:], pT[:], c1[:], AL.add)
                    nc.vector.tensor_scalar(pT[:], pT[:], 0.0, float(Hp - 1),
                                            AL.max, AL.min)
                    lt_t = pbig.tile([128, FULL], F32, tag="big")
                    nc.vector.tensor_scalar(lt_t[:], fT[:], 0.0, None, AL.max)
                    cb_t = pbig.tile([128, FULL], F32, tag="big")
                    nc.vector.tensor_scalar(cb_t[:], fT[:], 1.0, 0.0, AL.add,
                                            AL.max)
                    nc.vector.scalar_tensor_tensor(
                        wl, lt_t[:], 1.0, pT[:], AL.add, AL.subtract)
                    nc.vector.scalar_tensor_tensor(
                        wr, pT[:], 1.0, cb_t[:], AL.add, AL.subtract)

                    def xsl(t, base=0):
                        v = t[:] if not isinstance(t, AP) else t
                        return _ap(v, base, [v.ap[0], [128, NK], [1, 63]])

                    def ysl(t, base=0):
                        v = t[:] if not isinstance(t, AP) else t
                        return _ap(v, base + 64, [v.ap[0], [128, NK], [1, 63]])

                    YS = NK * 63
                    # x-side edge fold: wl += wr*lox ; wr -= wr*lox (lox = f<0)
                    lox = ph.tile([128, YS], F32, tag="hy")
                    nc.vector.tensor_scalar(lox[:], xsl(fT), 0.0, None, AL.is_lt)
                    tfx = ph.tile([128, YS], F32, tag="hy")
                    nc.vector.tensor_tensor(tfx[:], xsl(wlrv, FULL), lox[:],
                                            AL.mult)
                    nc.vector.tensor_tensor(xsl(wlrv), xsl(wlrv), tfx[:], AL.add)
                    nc.vector.tensor_tensor(xsl(wlrv, FULL), xsl(wlrv, FULL),
                                            tfx[:], AL.subtract)

                    # y-side folded pair weights v0/v1 (interleaved in vw)
                    fy_px = ysl(fT)
                    wyl, wyr = ysl(wlrv), ysl(wlrv, FULL)
                    hi = ph.tile([128, YS], F32, tag="hy")
                    nc.vector.tensor_scalar(hi[:], fy_px, float(Wp - 1), None,
                                            AL.is_ge)
                    lo = ph.tile([128, YS], F32, tag="hy")
                    nc.vector.tensor_scalar(lo[:], fy_px, -1.0, None, AL.is_le)
                    oh = ph.tile([128, YS], F32, tag="hy")
                    nc.vector.tensor_scalar(oh[:], hi[:], -1.0, 1.0, AL.mult,
                                            AL.add)
                    ol = ph.tile([128, YS], F32, tag="hy")
                    nc.vector.tensor_scalar(ol[:], lo[:], -1.0, 1.0, AL.mult,
                                            AL.add)
                    vwv = vw[:]
                    v0 = _ap(vwv, 0, [vwv.ap[0], [2, YS]])
                    v1 = _ap(vwv, 1, [vwv.ap[0], [2, YS]])
                    t2 = ph.tile([128, YS], F32, tag="hy")
                    nc.vector.tensor_tensor(oh[:], wyl, oh[:], AL.mult)
                    nc.vector.tensor_tensor(t2[:], wyr, lo[:], AL.mult)
                    nc.vector.tensor_tensor(v0, oh[:], t2[:], AL.add)
                    nc.vector.tensor_tensor(ol[:], wyr, ol[:], AL.mult)
                    nc.vector.tensor_tensor(hi[:], wyl, hi[:], AL.mult)
                    nc.vector.tensor_tensor(v1, ol[:], hi[:], AL.add)

            # ---------------- Phase C ----------------
            with (
                tc.tile_pool(name="gata", bufs=2) as pga,
                tc.tile_pool(name="wgp", bufs=2) as pwg,
                tc.tile_pool(name="vyp", bufs=2) as pvy,
                tc.tile_pool(name="xoffp", bufs=3) as pxo,
                tc.tile_pool(name="strip", bufs=2) as pstr,
                tc.tile_pool(name="outp", bufs=2) as pout,
                tc.tile_pool(name="psum_a", bufs=2, space="PSUM") as pma,
                tc.tile_pool(name="psum_b2", bufs=2, space="PSUM") as pmb,
                tc.tile_pool(name="psum_t", bufs=4, space="PSUM") as pstp,
            ):
                vwp = vw[:]
                wvw = wrapped[:]
                wsv = ws_t[:]
                nidx_regs = {}
                for nn in (8064, 896):
                    reg = nc.gpsimd.alloc_register(f"nidx{nn}")
                    nc.gpsimd.reg_mov(reg, nn)
                    nidx_regs[nn] = reg

                pend = {}

                def issue_gather(k):
                    vt = pga.tile([128, 63 * 256], BF16, tag="V")
                    vv = vt[:]
                    nc.gpsimd.dma_gather(
                        _ap(vv, 0, [vv.ap[0], [256, 63], [1, 256]]),
                        xpair_pairs,
                        _ap(wvw, k * 504, [wvw.ap[0], [1, 504]]),
                        8064, nidx_regs[8064], 2 * 2 * C, elem_step=2 * C,
                        single_packet=False)
                    pend[k] = vt

                issue_gather(0)
                for k in range(NK):
                    if k + 1 < NK:
                        issue_gather(k + 1)
                    xoffT = pxo.tile([128, NG * 640], BF16, tag="xoffT")
                    xv = xoffT[:]
                    nc.gpsimd.memset(
                        _ap(xv, 576, [xv.ap[0], [640, NG], [1, 64]]), 0.0)
                    vt = pend.pop(k)
                    vv = vt[:]
                    wg2 = pwg.tile([128, NG * 72], BF16, tag="wg2")
                    wg2v = wg2[:]
                    for jj in range(2):
                        vj = _ap(vwp, k * 126 + jj,
                                 [vwp.ap[0], [18, NG], [2, 9], [0, 2], [0, 2]])
                        ur = _ap(wlrv, k * 128,
                                 [wlrv.ap[0], [9, NG], [1, 9], [FULL, 2],
                                  [0, 2]])
                        nc.vector.tensor_tensor(
                            _ap(wg2v, 4 * jj,
                                [wg2v.ap[0], [72, NG], [8, 9], [2, 2],
                                 [1, 2]]),
                            vj, ur, AL.mult)
                    dat = _ap(vv, 0, [vv.ap[0], [256, 63], [64, 4],
                                      [2, 32], [1, 2]])
                    nc.vector.tensor_tensor(
                        dat, dat,
                        _ap(wg2v, 0, [wg2v.ap[0], [8, 63], [2, 4],
                                      [0, 32], [1, 2]]),
                        AL.mult)
                    vy = pvy.tile([128, 63 * 128], BF16, tag="vy")
                    vyv = vy[:]
                    nc.vector.tensor_tensor(
                        _ap(vyv, 0, [vyv.ap[0], [128, 63], [64, 2], [1, 64]]),
                        _ap(vv, 0, [vv.ap[0], [256, 63], [128, 2], [1, 64]]),
                        _ap(vv, 64, [vv.ap[0], [256, 63], [128, 2], [1, 64]]),
                        AL.add)
                    nc.vector.tensor_tensor(
                        _ap(xv, 0, [xv.ap[0], [640, NG], [64, 9], [1, 64]]),
                        _ap(vyv, 0, [vyv.ap[0], [1152, NG], [128, 9],
                                     [1, 64]]),
                        _ap(vyv, 64, [vyv.ap[0], [1152, NG], [128, 9],
                                      [1, 64]]),
                        AL.add)
                    strip = pstr.tile([128, 5 * 896], BF16, tag="strip")
                    sv = strip[:]
                    if k % 2 == 0:
                        nc.gpsimd.dma_gather(
                            _ap(sv, 0, [sv.ap[0], [896, 5], [1, 896]]),
                            xoffT[:],
                            _ap(wsv, 0, [wsv.ap[0], [1, 56]]),
                            896, nidx_regs[896], 640, transpose=True,
                            single_packet=False,
                            sbuf_tokens_per_rank=128,
                            sbuf_free_dim_per_rank=2 * 640)
                    else:
                        # PE-transpose route (Pool relief on odd k)
                        for g in range(NG):
                            for f in range(5):
                                pst4 = pstp.tile([128, 128], BF16, tag="pstr")
                                nc.tensor.transpose(
                                    pst4[:],
                                    _ap(xv, g * 640 + f * 128,
                                        [xv.ap[0], [1, 128]]),
                                    identb[:, :])
                                nc.scalar.copy(
                                    strip[:, f * 896 + g * 128:
                                          f * 896 + g * 128 + 128],
                                    pst4[:])
                    ps_a = pma.tile([64, 512], F32, tag="ps_a")
                    ps_b = pmb.tile([64, 384], F32, tag="ps_b")
                    for f in range(5):
                        kk = 128 if f < 4 else 64
                        nc.tensor.matmul(
                            ps_a[:], w_d_t[0:kk, f * 64:(f + 1) * 64],
                            strip[0:kk, f * 896:f * 896 + 512],
                            start=(f == 0), stop=(f == 4))
                    for f in range(5):
                        kk = 128 if f < 4 else 64
                        nc.tensor.matmul(
                            ps_b[:], w_d_t[0:kk, f * 64:(f + 1) * 64],
                            strip[0:kk, f * 896 + 512:f * 896 + 896],
                            start=(f == 0), stop=(f == 4))
                    out_t = pout.tile([64, 896], F32, tag="outt")
                    nc.scalar.activation(
                        out_t[:, 0:512], ps_a[:], AF.Identity, bias=b_d_t[:])
                    nc.scalar.activation(
                        out_t[:, 512:896], ps_b[:], AF.Identity, bias=b_d_t[:])
                    nc.sync.dma_start(
                        out_d[:, k * 896:(k + 1) * 896], out_t[:])
    nc.compile()
    return nc


# ---------------- host side ----------------

def _pixel_maps():
    cols = np.arange(NK * NG * 128)
    kg, p = cols // 128, cols % 128
    k, g = kg // NG, kg % NG
    c = k * 128 + p
    slot, q = c // 316, c % 316
    b = g + NG * slot
    raster = 316 * b + q
    valid = (slot < 6) & (b < NBLK) & (raster < P)
    return np.where(valid, raster, -1)


def _base_tables(r0, b_off):
    pn = np.array([-1.0, 0.0, 1.0], np.float32)
    pnx = np.repeat(pn, 3)
    pny = np.tile(pn, 3)
    base_pk = np.zeros((128, CW), np.float32)
    cc = np.arange(CW)
    slot, q = cc // 316, cc % 316
    for side in range(2):
        for n in range(9):
            for g in range(NG):
                b = g + NG * slot
                raster = 316 * b + q
                valid = (slot < 6) & (b < NBLK) & (raster < P)
                rr = np.where(valid, raster, 0)
                row_l, col_l = rr // w, rr % w
                if side == 0:
                    val = pnx[n] + (r0 + row_l) + 1.0
                else:
                    val = pny[n] + col_l + 1.0
                val = val + b_off[2 * n + side]
                base_pk[side * 64 + g * 9 + n] = np.where(valid, val, 0.0)
    return base_pk


def make_core_inputs(inputs, core):
    x = np.ascontiguousarray(inputs["x"], np.float32)
    w_off = np.ascontiguousarray(inputs["w_off"], np.float32)
    b_off = np.ascontiguousarray(inputs["b_off"], np.float32)
    w_d = np.ascontiguousarray(inputs["w_d"], np.float32)
    b_d = np.ascontiguousarray(inputs["b_d"], np.float32)
    bb, half = core // 2, core % 2
    r0 = half * R

    xp = np.pad(x[bb], ((0, 0), (1, 1), (1, 1)))          # [C, 162, 162]
    xpT = np.ascontiguousarray(xp.transpose(1, 2, 0))     # [162, 162, C]
    xpair = np.zeros((NPAIR, 2 * C), np.float32)
    a = xpair[:NPAIR_A].reshape(81, Wp, 2 * C)
    a[:, :, 0:C] = xpT[0:162:2]
    a[:, :, C:2 * C] = xpT[1:162:2]
    bpart = xpair[NPAIR_A:].reshape(80, Wp, 2 * C)
    bpart[:, :, 0:C] = xpT[1:161:2]
    bpart[:, :, C:2 * C] = xpT[2:162:2]
    xpair = xpair.astype(ml_dtypes.bfloat16)

    xr = x[bb][:, r0:r0 + 81, :]
    x_sb = np.ascontiguousarray(xr.reshape(C, 81 * W))
    x_sb_bf = x_sb.astype(ml_dtypes.bfloat16)
    x4 = np.ascontiguousarray(xr[:, 0:4, :].reshape(C, 4 * W))
    xc = np.zeros((C, 83 * 4), np.float32)
    xc[:, :81 * 4] = xr[:, :, 0:4].reshape(C, 81 * 4)

    # per-tap lhsT: out channel (side, n) at partition side*64+n
    w_taps = np.zeros((C, 9 * 128), np.float32)
    for t in range(9):
        for side in range(2):
            for n in range(9):
                w_taps[:, t * 128 + side * 64 + n] = \
                    w_off[2 * n + side, :, t // 3, t % 3]

    w_d_chunks = np.zeros((128, 5 * 64), np.float32)
    wd2 = w_d.reshape(64, 64, 9)
    for j in range(4):
        for rloc in range(128):
            n, cch = 2 * j + rloc // 64, rloc % 64
            w_d_chunks[rloc, j * 64:(j + 1) * 64] = wd2[:, cch, n]
    for rloc in range(64):
        w_d_chunks[rloc, 256:320] = wd2[:, rloc, 8]

    base_pk = _base_tables(r0, b_off)

    ws = np.zeros((128, 56), np.int16)
    for i in range(896):
        ws[i % 16, i // 16] = i
    ws[16:, :] = np.tile(ws[:16, :], (7, 1))

    return {
        "xpair_cl": xpair,
        "x_sb_in": x_sb_bf,
        "x4_in": x4,
        "xc_in": xc,
        "w_taps_in": w_taps.astype(ml_dtypes.bfloat16),
        "w_tapsf_in": w_taps,
        "w_d_in": w_d_chunks.astype(ml_dtypes.bfloat16),
        "b_d_in": b_d.reshape(64, 1).copy(),
        "base_pk_in": base_pk,
        "ident_in": np.eye(128, dtype=np.float32),
        "ws_in": ws,
    }


def reassemble(core_outs):
    rmap = _pixel_maps()
    valid = rmap >= 0
    rv = rmap[valid]
    out = np.zeros((B, 64, h, w), np.float32)
    for core, oc in enumerate(core_outs):
        bb, half = core // 2, core % 2
        r0 = half * R
        flat = np.zeros((64, P), np.float32)
        flat[:, rv] = oc[:, valid]
        out[bb, :, r0:r0 + R, :] = flat.reshape(64, R, w)
    return out


_NC_CACHE = {}


def kernel(**inputs) -> np.ndarray:
    from concourse.bass_utils import run_bass_kernel_spmd

    if "nc" not in _NC_CACHE:
        _NC_CACHE["nc"] = build_nc()
    nc = _NC_CACHE["nc"]
    in_maps = [make_core_inputs(inputs, core) for core in range(8)]
    res = run_bass_kernel_spmd(nc, in_maps, core_ids=list(range(8)))
    return reassemble([r["out_d"] for r in res.results])


# revision 4
# speedup vs baseline: 1.0026x; 1.0026x over previous
"""Deformable Conv2d (4,64,160,160) -> (4,64,158,158) on 8 trn2 NeuronCores.

v2: bf16 4-corner single gather from a dual-parity row-pair-interleaved DRAM
layout, fast-mode (2x) DVE bilinear combine with c-pair-replicated weights,
strips via SBUF-source transpose dma_gather (no PE transposes), bf16 convs
with fp32 edge re-pass for snap-boundary precision.

Sharding: core = (sample b = core//2, row-half = core%2); each core computes
79 output rows (12482 px) of one sample.

Layouts (per core):
  raster pixel p in [0, 12482): 40 conv blocks of 316 px (block 39: 158).
  packed col space: block b -> (g = b%7, slot = b//7), col = slot*316 + q.
  packed rows: side*64 + n*7 + g (side 0=x/1=y offsets, n tap, g group).
  px-layout: partition = packed col % 128 for col chunk k in [0,15).
  dual-parity pairs DRAM [26082, 128] bf16: entry (par,pr,y) = rows
  (2pr+par, 2pr+par+1) at col y, channels [row0 c*, row1 c*]; gather elem
  256 vals (step 128) = 4 bilinear corners [colY(2rows), colY+1(2rows)].
  gather idx = (s_x>>1)*162 + s_y + (s_x&1)*13122.
  output cols: (k*7+g)*128 + p  (packed-pixel order; host reassembles).
"""

import numpy as np
import ml_dtypes

import concourse.bass as bass
import concourse.mybir as mybir
import concourse.tile as tile
from concourse.bass import AP

F32 = mybir.dt.float32
BF16 = mybir.dt.bfloat16
I16 = mybir.dt.int16
I32 = mybir.dt.int32
AL = mybir.AluOpType
AF = mybir.ActivationFunctionType

B, C, H, W = 4, 64, 160, 160
Hp, Wp = H + 2, W + 2          # 162
h, w = H - 2, W - 2            # 158
R = h // 2                     # 79 rows per core
P = R * w                      # 12482 px per core
NBLK = 40
NG = 7
CW = 1920                      # packed cols
NK = 15
FULL = NK * 128                # 1920
RW = 42 * 316                  # 13272 raster staging cols
WPITCH = NK * 504              # 7560 wrapped idx pitch
NPAIR_A = 81 * Wp              # 13122
NPAIR = NPAIR_A + 80 * Wp      # 26082
OUTW = NK * 896                # 13440


def _ap(base: AP, extra_off, dims):
    return AP(base.tensor, base.offset + extra_off, dims)


def build_nc():
    from concourse import bacc
    nc = bacc.Bacc("TRN2")

    xpair_cl = nc.dram_tensor("xpair_cl", [NPAIR, 2 * C], BF16, kind="ExternalInput")
    x_sb_in = nc.dram_tensor("x_sb_in", [C, 81 * W], BF16, kind="ExternalInput")
    x4_in = nc.dram_tensor("x4_in", [C, 4 * W], F32, kind="ExternalInput")
    xc_in = nc.dram_tensor("xc_in", [C, 83 * 4], F32, kind="ExternalInput")
    w_taps_in = nc.dram_tensor("w_taps_in", [C, 9 * 128], BF16,
                               kind="ExternalInput")
    w_tapsf_in = nc.dram_tensor("w_tapsf_in", [C, 9 * 128], F32,
                                kind="ExternalInput")
    w_d_in = nc.dram_tensor("w_d_in", [128, 5 * 64], BF16, kind="ExternalInput")
    b_d_in = nc.dram_tensor("b_d_in", [64, 1], F32, kind="ExternalInput")
    base_pk_in = nc.dram_tensor("base_pk_in", [128, CW], F32, kind="ExternalInput")
    ident_in = nc.dram_tensor("ident_in", [128, 128], F32, kind="ExternalInput")
    ws_in = nc.dram_tensor("ws_in", [128, 56], I16, kind="ExternalInput")
    out_d = nc.dram_tensor("out_d", [64, OUTW], F32, kind="ExternalOutput")

    xpair_pairs = AP(xpair_cl[:].tensor, 0, [[2 * C, NPAIR - 1], [1, 4 * C]])

    with tile.TileContext(nc) as tc:
        with tc.tile_pool(name="persist", bufs=1) as pp:
            b_d_t = pp.tile([64, 1], F32, tag="bd")
            nc.sync.dma_start(b_d_t[:], b_d_in[:])
            w_d_t = pp.tile([128, 5 * 64], BF16, tag="wd")
            nc.sync.dma_start(w_d_t[:], w_d_in[:])
            ws_t = pp.tile([128, 56], I16, tag="ws")
            nc.sync.dma_start(ws_t[:], ws_in[:])
            identb = pp.tile([128, 128], BF16, tag="identb")
            nc.gpsimd.dma_start(identb[:], ident_in[:])

            wrapped = pp.tile([128, WPITCH], I16, tag="wrapped")
            wlr = pp.tile([128, 2 * FULL], F32, tag="wlr")
            vw = pp.tile([128, NK * 63 * 2], F32, tag="vw")
            wlrv = wlr[:]
            wl = _ap(wlrv, 0, [wlrv.ap[0], [1, FULL]])
            wr = _ap(wlrv, FULL, [wlrv.ap[0], [1, FULL]])

            with tc.tile_pool(name="mid", bufs=1) as pm:
                ident = pm.tile([128, 128], F32, tag="ident")
                nc.sync.dma_start(ident[:], ident_in[:])
                offs_pk = pm.tile([128, CW], F32, tag="offs_pk")
                nc.gpsimd.memset(offs_pk[:], 0.0)
                ov = offs_pk[:]
                ppk = ov.ap[0][0]

                # ---------------- Phase A: offset conv ----------------
                # packed rows now side*64 + g*9 + n; conv writes offs_pk
                # directly from psum (per-block g-specific lhsT layout).
                with (
                    tc.tile_pool(name="phaseA", bufs=1) as pa,
                    tc.tile_pool(name="psum_conv", bufs=2, space="PSUM") as pconv,
                ):
                    x_sb = pa.tile([C, 81 * W], BF16, tag="x_sb")
                    nc.sync.dma_start(x_sb[:], x_sb_in[:])
                    w_t = pa.tile([C, 9 * 128], BF16, tag="w_taps")
                    nc.sync.dma_start(w_t[:], w_taps_in[:])
                    x4 = pa.tile([C, 4 * W], F32, tag="x4")
                    nc.sync.dma_start(x4[:], x4_in[:])
                    xc = pa.tile([C, 83 * 4], F32, tag="xc")
                    nc.sync.dma_start(xc[:], xc_in[:])
                    w_tf = pa.tile([C, 9 * 128], F32, tag="w_tf")
                    nc.sync.dma_start(w_tf[:], w_tapsf_in[:])

                    stg = [pa.tile([128, 6 * 316], F32, tag=f"stg{g}",
                                   name=f"stg{g}") for g in range(NG)]
                    # unwritten tails (beyond last block per g)
                    nc.gpsimd.memset(stg[4][:, 5 * 316 + 158:6 * 316], 0.0)
                    nc.gpsimd.memset(stg[5][:, 5 * 316:6 * 316], 0.0)
                    nc.gpsimd.memset(stg[6][:, 5 * 316:6 * 316], 0.0)

                    xb = x_sb[:]
                    for blk in range(NBLK):
                        g, slot = blk % NG, blk // NG
                        rows = 2 if blk < NBLK - 1 else 1
                        npx = rows * w
                        ps = pconv.tile([128, 316], F32, tag="psc")
                        for t in range(9):
                            ki, kj = t // 3, t % 3
                            rhs = _ap(xb, (2 * blk + ki) * W + kj,
                                      [xb.ap[0], [W, rows], [1, w]])
                            nc.tensor.matmul(
                                ps[:, 0:npx],
                                w_t[:, t * 128:(t + 1) * 128],
                                rhs, start=(t == 0), stop=(t == 8))
                        for side in range(2):
                            nc.scalar.copy(
                                stg[g][side * 64:side * 64 + 9,
                                       slot * 316:slot * 316 + npx],
                                ps[side * 64:side * 64 + 9, 0:npx])

                    # fp32 edge row-pass (rows 0-1 = block 0, g=0)
                    x4v, xcv = x4[:], xc[:]
                    ps2 = pconv.tile([128, 316], F32, tag="ps_er")
                    for t in range(9):
                        ki, kj = t // 3, t % 3
                        rhs = _ap(x4v, ki * W + kj, [x4v.ap[0], [W, 2], [1, w]])
                        nc.tensor.matmul(
                            ps2[:], w_tf[:, t * 128:(t + 1) * 128], rhs,
                            start=(t == 0), stop=(t == 8))
                    for side in range(2):
                        nc.scalar.copy(
                            stg[0][side * 64:side * 64 + 9, 0:316],
                            ps2[side * 64:side * 64 + 9, :])

                    # fp32 edge col-pass (cols 0-1), per (g, parity)
                    for g in range(NG):
                        for par_ in range(2):
                            T = sum(1 for t in range(6)
                                    if 14 * t + 2 * g + par_ <= 78)
                            ps3 = pconv.tile([128, 12], F32, tag="ps_ec")
                            for t in range(9):
                                ki, kj = t // 3, t % 3
                                rhs = _ap(xcv, (2 * g + par_ + ki) * 4 + kj,
                                          [xcv.ap[0], [56, T], [1, 2]])
                                nc.tensor.matmul(
                                    ps3[:, 0:2 * T],
                                    w_tf[:, t * 128:(t + 1) * 128],
                                    rhs, start=(t == 0), stop=(t == 8))
                            sgv = stg[g][:]
                            sgp = sgv.ap[0][0]
                            p3v = ps3[:]
                            for side in range(2):
                                nc.scalar.copy(
                                    _ap(sgv, side * 64 * sgp + par_ * 158,
                                        [[sgp, 9], [316, T], [1, 2]]),
                                    _ap(p3v, side * 64 * p3v.ap[0][0],
                                        [[p3v.ap[0][0], 9], [2, T], [1, 2]]))

                    # scatter staging -> packed rows: 14 big DMAs
                    for g in range(NG):
                        sgv = stg[g][:]
                        sgp = sgv.ap[0][0]
                        for side in range(2):
                            eng = nc.sync if side == 0 else nc.scalar
                            eng.dma_start(
                                _ap(ov, (side * 64 + g * 9) * ppk,
                                    [[ppk, 9], [1, 6 * 316]]),
                                _ap(sgv, side * 64 * sgp,
                                    [[sgp, 9], [1, 6 * 316]]))

                # ---------------- Phase B ----------------
                with (
                    tc.tile_pool(name="phaseB", bufs=1) as pb,
                    tc.tile_pool(name="pbbig", bufs=4) as pbig,
                    tc.tile_pool(name="pbh", bufs=5) as ph,
                    tc.tile_pool(name="psum_b", bufs=4, space="PSUM") as psb,
                ):
                    pxy = pb.tile([128, CW], F32, tag="b_pxy")
                    fl = pb.tile([128, CW], F32, tag="b_fl")
                    with tc.tile_pool(name="pb1", bufs=1) as p1:
                        # B1: pair-idx pipeline (packed layout)
                        bpk = pbig.tile([128, CW], F32, tag="big")
                        nc.sync.dma_start(bpk[:], base_pk_in[:])
                        nc.vector.tensor_tensor(pxy[:], offs_pk[:], bpk[:], AL.add)
                        ci1 = p1.tile([128, CW], I32, tag="b_ci")
                        nc.vector.tensor_copy(ci1[:], pxy[:])
                        tmp = pbig.tile([128, CW], F32, tag="big")
                        nc.vector.tensor_copy(tmp[:], ci1[:])
                        nc.vector.tensor_tensor(fl[:], tmp[:], pxy[:], AL.is_gt)
                        nc.vector.tensor_tensor(fl[:], tmp[:], fl[:], AL.subtract)
                        s_q = p1.tile([128, CW], F32, tag="b_sq")
                        nc.vector.tensor_scalar(s_q[:], fl[:], 0.0, 160.0,
                                                AL.max, AL.min)
                        sx, sy = s_q[0:64, :], s_q[64:128, :]
                        # idx = (sx>>1)*162 + sy + par*13122
                        #     = 81*sx + sy + (13122-81)*par,  par = sx mod 2
                        # (sx is an exact f32 integer, so mod 2 is exact)
                        syc = p1.tile([64, CW], F32, tag="b_syc")
                        nc.scalar.copy(syc[:], sy)
                        th = p1.tile([64, CW], F32, tag="b_th")
                        nc.vector.tensor_scalar(th[:], sx, 0.5, None, AL.mult)
                        ci2 = p1.tile([64, CW], I32, tag="b_ci2")
                        nc.vector.tensor_copy(ci2[:], th[:])
                        tm2 = p1.tile([64, CW], F32, tag="b_tm2")
                        nc.vector.tensor_copy(tm2[:], ci2[:])
                        hx = p1.tile([64, CW], F32, tag="b_hx")
                        nc.vector.tensor_tensor(hx[:], tm2[:], th[:], AL.is_gt)
                        nc.vector.tensor_tensor(hx[:], tm2[:], hx[:], AL.subtract)
                        par = p1.tile([64, CW], F32, tag="b_par")
                        nc.vector.scalar_tensor_tensor(
                            par[:], hx[:], -2.0, sx, AL.mult, AL.add)
                        t3 = p1.tile([64, CW], F32, tag="b_t3")
                        nc.vector.scalar_tensor_tensor(
                            t3[:], sx, 81.0, syc[:], AL.mult, AL.add)
                        idxq = p1.tile([64, CW], F32, tag="b_idxq")
                        nc.vector.scalar_tensor_tensor(
                            idxq[:], par[:], float(NPAIR_A - 81), t3[:],
                            AL.mult, AL.add)

                        # B2: wrapped idx via [64,16] PE transposes, batched
                        # per-k psum + one Act copy + per-k replication
                        idv = idxq[:]
                        wv = wrapped[:]
                        wpp = wv.ap[0][0]
                        for k in range(NK):
                            pst = psb.tile([16, 512], F32, tag="ps_wrap")
                            pv = pst[:]
                            for sl in range(8):
                                nc.tensor.transpose(
                                    _ap(pv, sl * 64, [pv.ap[0], [1, 64]]),
                                    _ap(idv, 16 * (k * 8 + sl),
                                        [idv.ap[0], [1, 16]]),
                                    ident[0:64, 0:64])
                            nc.scalar.copy(
                                _ap(wv, k * 504,
                                    [[wpp, 16], [1, 8], [8, 9], [72, 7]]),
                                _ap(pv, 0,
                                    [[pv.ap[0][0], 16], [64, 8], [1, 9],
                                     [9, 7]]))
                            for i, rep in enumerate((16, 32, 64)):
                                eng = nc.sync if i % 2 == 0 else nc.scalar
                                eng.dma_start(
                                    _ap(wv, k * 504 + rep * wpp,
                                        [[wpp, rep], [1, 504]]),
                                    _ap(wv, k * 504, [[wpp, rep], [1, 504]]))

                    # B3: transpose positions+floors to px layout
                    pT = pb.tile([128, FULL], F32, tag="b_pT")
                    fT = pb.tile([128, FULL], F32, tag="b_fT")
                    for src, dst in ((pxy, pT), (fl, fT)):
                        sv_ = src[:]
                        for k in range(NK):
                            pst2 = psb.tile([128, 128], F32, tag="ps_ot")
                            nc.tensor.transpose(
                                pst2[:], _ap(sv_, 128 * k, [sv_.ap[0], [1, 128]]),
                                ident[:, :])
                            nc.scalar.copy(dst[:, k * 128:(k + 1) * 128], pst2[:])

                    # B4: weights (px layout); p=pT, f=fT
                    c1 = pbig.tile([128, FULL], F32, tag="big")
                    nc.vector.tensor_scalar(c1[:], pT[:], 1.0, None, AL.is_lt)
                    c2 = pbig.tile([128, FULL], F32, tag="big")
                    nc.vector.tensor_scalar(c2[:], pT[:], float(H), None, AL.is_gt)
                    nc.vector.tensor_tensor(c2[:], c1[:], c2[:], AL.max)
                    nc.vector.tensor_tensor(c1[:], fT[:], pT[:], AL.subtract)
                    nc.vector.tensor_tensor(c1[:], c2[:], c1[:], AL.mult)
                    nc.vector.tensor_tensor(pT[:], pT[:], c1[:], AL.add)
                    nc.vector.tensor_scalar(pT[:], pT[:], 0.0, float(Hp - 1),
                                            AL.max, AL.min)
                    lt_t = pbig.tile([128, FULL], F32, tag="big")
                    nc.scalar.activation(lt_t[:], fT[:], AF.Relu)
                    cb_t = pbig.tile([128, FULL], F32, tag="big")
                    nc.scalar.activation(cb_t[:], fT[:], AF.Relu, bias=1.0)
                    nc.vector.scalar_tensor_tensor(
                        wl, lt_t[:], 1.0, pT[:], AL.add, AL.subtract)
                    nc.vector.scalar_tensor_tensor(
                        wr, pT[:], 1.0, cb_t[:], AL.add, AL.subtract)

                    def xsl(t, base=0):
                        v = t[:] if not isinstance(t, AP) else t
                        return _ap(v, base, [v.ap[0], [128, NK], [1, 63]])

                    def ysl(t, base=0):
                        v = t[:] if not isinstance(t, AP) else t
                        return _ap(v, base + 64, [v.ap[0], [128, NK], [1, 63]])

                    YS = NK * 63
                    # x-side edge fold: wl += wr*lox ; wr -= wr*lox (lox = f<0)
                    lox = ph.tile([128, YS], F32, tag="hy")
                    nc.vector.tensor_scalar(lox[:], xsl(fT), 0.0, None, AL.is_lt)
                    tfx = ph.tile([128, YS], F32, tag="hy")
                    nc.vector.tensor_tensor(tfx[:], xsl(wlrv, FULL), lox[:],
                                            AL.mult)
                    nc.vector.tensor_tensor(xsl(wlrv), xsl(wlrv), tfx[:], AL.add)
                    nc.vector.tensor_tensor(xsl(wlrv, FULL), xsl(wlrv, FULL),
                                            tfx[:], AL.subtract)

                    # y-side folded pair weights v0/v1 (interleaved in vw)
                    fy_px = ysl(fT)
                    wyl, wyr = ysl(wlrv), ysl(wlrv, FULL)
                    hi = ph.tile([128, YS], F32, tag="hy")
                    nc.vector.tensor_scalar(hi[:], fy_px, float(Wp - 1), None,
                                            AL.is_ge)
                    lo = ph.tile([128, YS], F32, tag="hy")
                    nc.vector.tensor_scalar(lo[:], fy_px, -1.0, None, AL.is_le)
                    oh = ph.tile([128, YS], F32, tag="hy")
                    nc.vector.tensor_scalar(oh[:], hi[:], -1.0, 1.0, AL.mult,
                                            AL.add)
                    ol = ph.tile([128, YS], F32, tag="hy")
                    nc.vector.tensor_scalar(ol[:], lo[:], -1.0, 1.0, AL.mult,
                                            AL.add)
                    vwv = vw[:]
                    v0 = _ap(vwv, 0, [vwv.ap[0], [2, YS]])
                    v1 = _ap(vwv, 1, [vwv.ap[0], [2, YS]])
                    t2 = ph.tile([128, YS], F32, tag="hy")
                    nc.vector.tensor_tensor(oh[:], wyl, oh[:], AL.mult)
                    nc.vector.tensor_tensor(t2[:], wyr, lo[:], AL.mult)
                    nc.vector.tensor_tensor(v0, oh[:], t2[:], AL.add)
                    nc.vector.tensor_tensor(ol[:], wyr, ol[:], AL.mult)
                    nc.vector.tensor_tensor(hi[:], wyl, hi[:], AL.mult)
                    nc.vector.tensor_tensor(v1, ol[:], hi[:], AL.add)

            # ---------------- Phase C ----------------
            with (
                tc.tile_pool(name="gata", bufs=2) as pga,
                tc.tile_pool(name="wgp", bufs=2) as pwg,
                tc.tile_pool(name="vyp", bufs=2) as pvy,
                tc.tile_pool(name="xoffp", bufs=3) as pxo,
                tc.tile_pool(name="strip", bufs=2) as pstr,
                tc.tile_pool(name="outp", bufs=2) as pout,
                tc.tile_pool(name="psum_a", bufs=2, space="PSUM") as pma,
                tc.tile_pool(name="psum_b2", bufs=2, space="PSUM") as pmb,
                tc.tile_pool(name="psum_t", bufs=4, space="PSUM") as pstp,
            ):
                vwp = vw[:]
                wvw = wrapped[:]
                wsv = ws_t[:]
                nidx_regs = {}
                for nn in (8064, 896):
                    reg = nc.gpsimd.alloc_register(f"nidx{nn}")
                    nc.gpsimd.reg_mov(reg, nn)
                    nidx_regs[nn] = reg

                pend = {}

                def issue_gather(k):
                    vt = pga.tile([128, 63 * 256], BF16, tag="V")
                    vv = vt[:]
                    nc.gpsimd.dma_gather(
                        _ap(vv, 0, [vv.ap[0], [256, 63], [1, 256]]),
                        xpair_pairs,
                        _ap(wvw, k * 504, [wvw.ap[0], [1, 504]]),
                        8064, nidx_regs[8064], 2 * 2 * C, elem_step=2 * C,
                        single_packet=False)
                    pend[k] = vt

                issue_gather(0)
                for k in range(NK):
                    if k + 1 < NK:
                        issue_gather(k + 1)
                    xoffT = pxo.tile([128, NG * 640], BF16, tag="xoffT")
                    xv = xoffT[:]
                    nc.gpsimd.memset(
                        _ap(xv, 576, [xv.ap[0], [640, NG], [1, 64]]), 0.0)
                    vt = pend.pop(k)
                    vv = vt[:]
                    wg2 = pwg.tile([128, NG * 72], BF16, tag="wg2")
                    wg2v = wg2[:]
                    for jj in range(2):
                        vj = _ap(vwp, k * 126 + jj,
                                 [vwp.ap[0], [18, NG], [2, 9], [0, 2], [0, 2]])
                        ur = _ap(wlrv, k * 128,
                                 [wlrv.ap[0], [9, NG], [1, 9], [FULL, 2],
                                  [0, 2]])
                        nc.vector.tensor_tensor(
                            _ap(wg2v, 4 * jj,
                                [wg2v.ap[0], [72, NG], [8, 9], [2, 2],
                                 [1, 2]]),
                            vj, ur, AL.mult)
                    dat = _ap(vv, 0, [vv.ap[0], [256, 63], [64, 4],
                                      [2, 32], [1, 2]])
                    nc.vector.tensor_tensor(
                        dat, dat,
                        _ap(wg2v, 0, [wg2v.ap[0], [8, 63], [2, 4],
                                      [0, 32], [1, 2]]),
                        AL.mult)
                    vy = pvy.tile([128, 63 * 128], BF16, tag="vy")
                    vyv = vy[:]
                    nc.vector.tensor_tensor(
                        _ap(vyv, 0, [vyv.ap[0], [128, 63], [64, 2], [1, 64]]),
                        _ap(vv, 0, [vv.ap[0], [256, 63], [128, 2], [1, 64]]),
                        _ap(vv, 64, [vv.ap[0], [256, 63], [128, 2], [1, 64]]),
                        AL.add)
                    nc.vector.tensor_tensor(
                        _ap(xv, 0, [xv.ap[0], [640, NG], [64, 9], [1, 64]]),
                        _ap(vyv, 0, [vyv.ap[0], [1152, NG], [128, 9],
                                     [1, 64]]),
                        _ap(vyv, 64, [vyv.ap[0], [1152, NG], [128, 9],
                                      [1, 64]]),
                        AL.add)
                    strip = pstr.tile([128, 5 * 896], BF16, tag="strip")
                    sv = strip[:]
                    if k % 2 == 0:
                        nc.gpsimd.dma_gather(
                            _ap(sv, 0, [sv.ap[0], [896, 5], [1, 896]]),
                            xoffT[:],
                            _ap(wsv, 0, [wsv.ap[0], [1, 56]]),
                            896, nidx_regs[896], 640, transpose=True,
                            single_packet=False,
                            sbuf_tokens_per_rank=128,
                            sbuf_free_dim_per_rank=2 * 640)
                    else:
                        # PE-transpose route (Pool relief on odd k)
                        for g in range(NG):
                            for f in range(5):
                                pst4 = pstp.tile([128, 128], BF16, tag="pstr")
                                nc.tensor.transpose(
                                    pst4[:],
                                    _ap(xv, g * 640 + f * 128,
                                        [xv.ap[0], [1, 128]]),
                                    identb[:, :])
                                nc.scalar.copy(
                                    strip[:, f * 896 + g * 128:
                                          f * 896 + g * 128 + 128],
                                    pst4[:])
                    ps_a = pma.tile([64, 512], F32, tag="ps_a")
                    ps_b = pmb.tile([64, 384], F32, tag="ps_b")
                    for f in range(5):
                        kk = 128 if f < 4 else 64
                        nc.tensor.matmul(
                            ps_a[:], w_d_t[0:kk, f * 64:(f + 1) * 64],
                            strip[0:kk, f * 896:f * 896 + 512],
                            start=(f == 0), stop=(f == 4))
                    for f in range(5):
                        kk = 128 if f < 4 else 64
                        nc.tensor.matmul(
                            ps_b[:], w_d_t[0:kk, f * 64:(f + 1) * 64],
                            strip[0:kk, f * 896 + 512:f * 896 + 896],
                            start=(f == 0), stop=(f == 4))
                    out_t = pout.tile([64, 896], F32, tag="outt")
                    nc.scalar.activation(
                        out_t[:, 0:512], ps_a[:], AF.Identity, bias=b_d_t[:])
                    nc.scalar.activation(
                        out_t[:, 512:896], ps_b[:], AF.Identity, bias=b_d_t[:])
                    nc.sync.dma_start(
                        out_d[:, k * 896:(k + 1) * 896], out_t[:])
    nc.compile()
    return nc


# ---------------- host side ----------------

def _pixel_maps():
    cols = np.arange(NK * NG * 128)
    kg, p = cols // 128, cols % 128
    k, g = kg // NG, kg % NG
    c = k * 128 + p
    slot, q = c // 316, c % 316
    b = g + NG * slot
    raster = 316 * b + q
    valid = (slot < 6) & (b < NBLK) & (raster < P)
    return np.where(valid, raster, -1)


def _base_tables(r0, b_off):
    pn = np.array([-1.0, 0.0, 1.0], np.float32)
    pnx = np.repeat(pn, 3)
    pny = np.tile(pn, 3)
    base_pk = np.zeros((128, CW), np.float32)
    cc = np.arange(CW)
    slot, q = cc // 316, cc % 316
    for side in range(2):
        for n in range(9):
            for g in range(NG):
                b = g + NG * slot
                raster = 316 * b + q
                valid = (slot < 6) & (b < NBLK) & (raster < P)
                rr = np.where(valid, raster, 0)
                row_l, col_l = rr // w, rr % w
                if side == 0:
                    val = pnx[n] + (r0 + row_l) + 1.0
                else:
                    val = pny[n] + col_l + 1.0
                val = val + b_off[2 * n + side]
                base_pk[side * 64 + g * 9 + n] = np.where(valid, val, 0.0)
    return base_pk


def make_core_inputs(inputs, core):
    x = np.ascontiguousarray(inputs["x"], np.float32)
    w_off = np.ascontiguousarray(inputs["w_off"], np.float32)
    b_off = np.ascontiguousarray(inputs["b_off"], np.float32)
    w_d = np.ascontiguousarray(inputs["w_d"], np.float32)
    b_d = np.ascontiguousarray(inputs["b_d"], np.float32)
    bb, half = core // 2, core % 2
    r0 = half * R

    xp = np.pad(x[bb], ((0, 0), (1, 1), (1, 1)))          # [C, 162, 162]
    xpT = np.ascontiguousarray(xp.transpose(1, 2, 0))     # [162, 162, C]
    xpair = np.zeros((NPAIR, 2 * C), np.float32)
    a = xpair[:NPAIR_A].reshape(81, Wp, 2 * C)
    a[:, :, 0:C] = xpT[0:162:2]
    a[:, :, C:2 * C] = xpT[1:162:2]
    bpart = xpair[NPAIR_A:].reshape(80, Wp, 2 * C)
    bpart[:, :, 0:C] = xpT[1:161:2]
    bpart[:, :, C:2 * C] = xpT[2:162:2]
    xpair = xpair.astype(ml_dtypes.bfloat16)

    xr = x[bb][:, r0:r0 + 81, :]
    x_sb = np.ascontiguousarray(xr.reshape(C, 81 * W))
    x_sb_bf = x_sb.astype(ml_dtypes.bfloat16)
    x4 = np.ascontiguousarray(xr[:, 0:4, :].reshape(C, 4 * W))
    xc = np.zeros((C, 83 * 4), np.float32)
    xc[:, :81 * 4] = xr[:, :, 0:4].reshape(C, 81 * 4)

    # per-tap lhsT: out channel (side, n) at partition side*64+n
    w_taps = np.zeros((C, 9 * 128), np.float32)
    for t in range(9):
        for side in range(2):
            for n in range(9):
                w_taps[:, t * 128 + side * 64 + n] = \
                    w_off[2 * n + side, :, t // 3, t % 3]

    w_d_chunks = np.zeros((128, 5 * 64), np.float32)
    wd2 = w_d.reshape(64, 64, 9)
    for j in range(4):
        for rloc in range(128):
            n, cch = 2 * j + rloc // 64, rloc % 64
            w_d_chunks[rloc, j * 64:(j + 1) * 64] = wd2[:, cch, n]
    for rloc in range(64):
        w_d_chunks[rloc, 256:320] = wd2[:, rloc, 8]

    base_pk = _base_tables(r0, b_off)

    ws = np.zeros((128, 56), np.int16)
    for i in range(896):
        ws[i % 16, i // 16] = i
    ws[16:, :] = np.tile(ws[:16, :], (7, 1))

    return {
        "xpair_cl": xpair,
        "x_sb_in": x_sb_bf,
        "x4_in": x4,
        "xc_in": xc,
        "w_taps_in": w_taps.astype(ml_dtypes.bfloat16),
        "w_tapsf_in": w_taps,
        "w_d_in": w_d_chunks.astype(ml_dtypes.bfloat16),
        "b_d_in": b_d.reshape(64, 1).copy(),
        "base_pk_in": base_pk,
        "ident_in": np.eye(128, dtype=np.float32),
        "ws_in": ws,
    }


def reassemble(core_outs):
    rmap = _pixel_maps()
    valid = rmap >= 0
    rv = rmap[valid]
    out = np.zeros((B, 64, h, w), np.float32)
    for core, oc in enumerate(core_outs):
        bb, half = core // 2, core % 2
        r0 = half * R
        flat = np.zeros((64, P), np.float32)
        flat[:, rv] = oc[:, valid]
        out[bb, :, r0:r0 + R, :] = flat.reshape(64, R, w)
    return out


_NC_CACHE = {}


def kernel(**inputs) -> np.ndarray:
    from concourse.bass_utils import run_bass_kernel_spmd

    if "nc" not in _NC_CACHE:
        _NC_CACHE["nc"] = build_nc()
    nc = _NC_CACHE["nc"]
    in_maps = [make_core_inputs(inputs, core) for core in range(8)]
    res = run_bass_kernel_spmd(nc, in_maps, core_ids=list(range(8)))
    return reassemble([r["out_d"] for r in res.results])
